# revision 52
# baseline (speedup 1.0000x reference)
"""RWKV6 block (nn_Block_14602888806424) on 8 Trainium2 NeuronCores.

Token-sharded (sequence-parallel): each core owns 512 tokens (B=2 x 4
blocks); matmuls/LNs/mixing are token-local in channel-major layout.
r/k/w/v are redistributed head-sharded via bf16 AllToAll around the
chunked (L=128) WKV linear-attention scan (4 heads/core); the WKV inner
loop produces channel-major y directly (no output transposes) and runs a
batched GroupNorm per 512-token block with rsqrt = exp(-0.5*ln(var+eps))
so the whole scan stays on one activation table. A second bf16 AllToAll
returns gn(y); silu/sigmoid are computed as x/(1+exp(-x)) to avoid
activation-table swaps. A small AllGather carries the 1-token boundary
halo for the second token-shift. Projections/FFN in bf16 with f32 PSUM
accumulation; all streamed weights host-pre-tiled so every DMA is
partition-contiguous (4KB/partition lines).
"""

import sys
import numpy as np

sys.path.insert(0, "/opt/trn_rl_repo")

import concourse.bass as bass
import concourse.bacc as bacc
import concourse.mybir as mybir
import concourse.tile as tile
from concourse import bass_utils

F32 = mybir.dt.float32
BF16 = mybir.dt.bfloat16
NP_BF16 = mybir.dt.np(BF16)
AF = mybir.ActivationFunctionType
ALU = mybir.AluOpType

B, T, C, H, N, FF = 2, 2048, 2048, 32, 64, 7168
D_MIX, D_DECAY = 32, 64
EPS_LN = 1e-5
EPS_LNX = 1e-5 * 8.0**2
NCORE = 8
TB = 512
KC = C // 128          # 16
KF = FF // 128         # 56
LCH = 256              # channels per core (4 heads)
RG = [list(range(NCORE))]


def build_program():
    nc = bacc.Bacc("TRN2", target_bir_lowering=False, debug=False,
                   num_devices=NCORE, enable_asserts=False)

    def din(name, shape, dt=F32):
        return nc.dram_tensor(name, list(shape), dt, kind="ExternalInput").ap()

    xT = din("xT", (C, TB + 1))
    halo_mask = din("halo_mask", (128, 1))
    sel_prev = din("sel_prev", (NCORE, 1), BF16)
    u_loc = din("u_loc", (128, 2))
    lnx_cols = din("lnx_cols", (128, 4))
    ln1_wb = din("ln1_wb", (C, 2))
    ln2_wb = din("ln2_wb", (C, 2))
    tm_maaT = din("tm_maaT", (C, 6))
    cm_maaT = din("cm_maaT", (C, 2))
    td_col = din("td_col", (C, 1))
    ident = din("ident", (128, 128))
    ident_bf = din("ident_bf", (128, 128), BF16)
    mask_su = din("mask_su", (128, 128))
    blk2h = din("blk2h", (128, 2), BF16)
    sel2h = din("sel2h", (2, 128), BF16)
    maa_w1 = din("maa_w1", (C, 5 * D_MIX), BF16)
    maa_w2s = din("maa_w2s", (5 * 4 * 32, 4 * 128), BF16)
    td_w1 = din("td_w1", (C, D_DECAY), BF16)
    td_w2l = din("td_w2l", (64, 2 * 128), BF16)
    td_loc = din("td_loc", (128, 2))
    Wp = {k: din(f"W{k}_p", (C, KC * 128), BF16)
          for k in ["r", "k", "g", "o", "cr"]}
    Wv_p = din("Wv_p", (4 * 4 * 128, 4 * 512), BF16)
    Wck_p = din("Wck_p", (KF * 128, KC * 128), BF16)
    Wcv_p = din("Wcv_p", (KC * 128, KF * 128), BF16)

    outT = nc.dram_tensor("out", [C, TB], F32, kind="ExternalOutput").ap()

    with tile.TileContext(nc) as tc:
        import contextlib
        with contextlib.ExitStack() as ctx:
            dram = ctx.enter_context(tc.tile_pool(name="dram", bufs=1,
                                                  space="DRAM"))
            cpool = ctx.enter_context(tc.tile_pool(name="const", bufs=1))
            big = ctx.enter_context(tc.tile_pool(name="big", bufs=1))
            wstr = ctx.enter_context(tc.tile_pool(name="wstr", bufs=3))
            sc = ctx.enter_context(tc.tile_pool(name="scratch", bufs=2))
            scw = ctx.enter_context(tc.tile_pool(name="scw", bufs=1))
            lnp = ctx.enter_context(tc.tile_pool(name="lnp", bufs=1))
            ps = ctx.enter_context(
                tc.tile_pool(name="psum", bufs=6, space="PSUM"))

            def pp(p_, f_):
                return ps.tile([p_, f_], F32, tag="pp", name="pp")

            def ppb(p_, f_):
                return ps.tile([p_, f_], BF16, tag="pp", name="pp")

            # ---- DRAM internals (collective payloads in bf16) ----
            agt_in = dram.tile([64, TB], BF16, tag="agt_in")
            agt_out = dram.tile([NCORE, 64, TB], BF16, tag="agt_out",
                                addr_space="Shared")
            a2ark_in = dram.tile([NCORE, 2, LCH, TB], BF16, tag="a2ark_in")
            a2ark_out = dram.tile([NCORE, 2, LCH, TB], BF16,
                                  tag="a2ark_out")
            a2v_in = dram.tile([NCORE, TB, LCH], BF16, tag="a2v_in")
            a2v_out = dram.tile([NCORE, TB, LCH], BF16, tag="a2v_out")
            a2b_in = dram.tile([NCORE, LCH, TB], BF16, tag="a2b_in")
            a2b_out = dram.tile([NCORE, LCH, TB], BF16, tag="a2b_out")
            ag_in = dram.tile([1, C], BF16, tag="ag_in")
            ag_out = dram.tile([NCORE, C], BF16, tag="ag_out",
                               addr_space="Shared")
            x2d = dram.tile([C, TB + 1], F32, tag="x2d")

            # ---- constants ----
            def cload(name, src, shape, dt=F32, rearr=None, **kw):
                t = cpool.tile(list(shape), dt, tag=name)
                nc.sync.dma_start(t[:], src if rearr is None
                                  else src.rearrange(rearr, **kw))
                return t

            c_ln1 = cload("c_ln1", ln1_wb, (128, KC, 2), F32,
                          "(k p) f -> p k f", p=128)
            c_ln2 = cload("c_ln2", ln2_wb, (128, KC, 2), F32,
                          "(k p) f -> p k f", p=128)
            c_tm = cload("c_tm", tm_maaT, (128, KC, 6), F32,
                         "(k p) f -> p k f", p=128)
            c_cm = cload("c_cm", cm_maaT, (128, KC, 2), F32,
                         "(k p) f -> p k f", p=128)
            c_hm = cload("c_hm", halo_mask, (128, 1))
            c_sel = cload("c_sel", sel_prev, (NCORE, 1), BF16)
            c_u = cload("c_u", u_loc, (128, 2))
            c_lnxc = cload("c_lnxc", lnx_cols, (128, 4))
            c_idb = cload("c_idb", ident_bf, (128, 128), BF16)
            c_msk = cload("c_msk", mask_su, (128, 128))
            c_blk2 = cload("c_blk2", blk2h, (128, 2), BF16)
            c_sel2 = cload("c_sel2", sel2h, (2, 128), BF16)
            c_w1 = cload("c_w1", maa_w1, (128, KC, 5 * D_MIX), BF16,
                         "(k p) f -> p k f", p=128)
            c_td1 = cload("c_td1", td_w1, (128, KC, D_DECAY), BF16,
                          "(k p) f -> p k f", p=128)
            c_td2l = cload("c_td2l", td_w2l, (64, 2, 128), BF16,
                           "p (h f) -> p h f", h=2)
            c_tdl = cload("c_tdl", td_loc, (128, 2))
            ones_col = cpool.tile([128, 1], F32, tag="ones_col")
            nc.vector.memset(ones_col[:], 1.0)
            ones_colb = cpool.tile([128, 1], BF16, tag="ones_colb")
            nc.vector.memset(ones_colb[:], 1.0)
            ones_row = cpool.tile([1, 128], F32, tag="ones_row")
            nc.vector.memset(ones_row[:], 1.0)
            for _cv in (EPS_LN, EPS_LNX):
                cvt = cpool.tile([128, 1], F32, tag=f"cv{_cv}", name="cvt")
                nc.vector.memset(cvt[:], _cv)
                nc.const_aps.aps[(F32, _cv)] = cvt[:]

            # ---- persistent SBUF ----
            ht = big.tile([128, KC, TB + 1], BF16, tag="ht")
            xx = big.tile([128, KC, TB], BF16, tag="xx")      # later xk2
            gsb = big.tile([128, KC, TB], BF16, tag="gsb")    # later xr2

            # ============ layernorm over TB+1 cols ============
            def layer_norm_ext(src_fn, dst_view, wb, eps):
                """src_fn(k)->(128,TB+1) f32 AP-producing fn; called twice."""
                psA, psB = pp(1, TB), pp(1, 1)
                psA2, psB2 = pp(1, TB), pp(1, 1)
                for k in range(KC):
                    s = src_fn(k)
                    sq = sc.tile([128, TB + 1], F32, tag="e2")
                    nc.scalar.activation(sq[:], s[:], AF.Square)
                    st, sp = (k == 0), (k == KC - 1)
                    nc.tensor.matmul(psA[:], ones_col[:], s[:, 0:TB],
                                     start=st, stop=sp)
                    nc.tensor.matmul(psB[:], ones_col[:], s[:, TB:TB + 1],
                                     start=st, stop=sp)
                    nc.tensor.matmul(psA2[:], ones_col[:], sq[:, 0:TB],
                                     start=st, stop=sp)
                    nc.tensor.matmul(psB2[:], ones_col[:], sq[:, TB:TB + 1],
                                     start=st, stop=sp)
                stats = lnp.tile([1, 2 * (TB + 1)], F32, tag="ln_stats")
                mean, msq = stats[:, 0:TB + 1], stats[:, TB + 1:]
                nc.scalar.activation(mean[:, 0:TB], psA[:], AF.Copy,
                                     scale=1.0 / C)
                nc.scalar.activation(mean[:, TB:TB + 1], psB[:], AF.Copy,
                                     scale=1.0 / C)
                nc.scalar.activation(msq[:, 0:TB], psA2[:], AF.Copy,
                                     scale=1.0 / C)
                nc.scalar.activation(msq[:, TB:TB + 1], psB2[:], AF.Copy,
                                     scale=1.0 / C)
                wk = lnp.tile([1, TB + 1], F32, tag="ln_work")
                nc.vector.tensor_mul(wk[:], mean[:], mean[:])
                nc.vector.tensor_sub(wk[:], msq[:], wk[:])
                nc.scalar.activation(wk[:], wk[:], AF.Ln, bias=eps)
                nc.scalar.activation(wk[:], wk[:], AF.Exp, scale=-0.5)
                bmp, bmp2 = pp(128, TB), pp(128, 1)
                bip, bip2 = pp(128, TB), pp(128, 1)
                nc.tensor.matmul(bmp[:], ones_row[:], mean[:, 0:TB],
                                 start=True, stop=True)
                nc.tensor.matmul(bmp2[:], ones_row[:], mean[:, TB:TB + 1],
                                 start=True, stop=True)
                nc.tensor.matmul(bip[:], ones_row[:], wk[:, 0:TB],
                                 start=True, stop=True)
                nc.tensor.matmul(bip2[:], ones_row[:], wk[:, TB:TB + 1],
                                 start=True, stop=True)
                bc = lnp.tile([128, 2 * (TB + 1)], BF16, tag="ln_bc")
                bm, bi = bc[:, 0:TB + 1], bc[:, TB + 1:]
                nc.vector.tensor_copy(bm[:, 0:TB], bmp[:])
                nc.vector.tensor_copy(bm[:, TB:TB + 1], bmp2[:])
                nc.vector.tensor_copy(bi[:, 0:TB], bip[:])
                nc.vector.tensor_copy(bi[:, TB:TB + 1], bip2[:])
                for k in range(KC):
                    s = src_fn(k)
                    t = sc.tile([128, TB + 1], F32, tag="e2")
                    nc.vector.tensor_sub(t[:], s[:], bm[:])
                    nc.vector.tensor_mul(t[:], t[:], bi[:])
                    d = dst_view(k)
                    nc.vector.tensor_scalar(d, t[:], wb[:, k, 0:1],
                                            wb[:, k, 1:2], ALU.mult, ALU.add)
                    nc.vector.tensor_scalar(d[:, 0:1], d[:, 0:1], c_hm[:],
                                            None, ALU.mult)

            def xt_src(k):
                t = sc.tile([128, TB + 1], F32, tag="e1", bufs=2)
                nc.sync.dma_start(t[:], xT[128 * k:128 * (k + 1), :])
                return t

            # ============ phase A: ln1 + shift ============
            layer_norm_ext(xt_src, lambda k: ht[:, k, :], c_ln1, EPS_LN)
            for k in range(KC):
                nc.vector.tensor_sub(xx[:, k, :], ht[:, k, 0:TB],
                                     ht[:, k, 1:TB + 1])

            # ============ phase B: maa ============
            aps1, aps2 = pp(128, TB), pp(32, TB)
            for k in range(KC):
                xxx = sc.tile([128, TB], BF16, tag="xxx")
                nc.vector.scalar_tensor_tensor(
                    xxx[:], xx[:, k, :], c_tm[:, k, 0:1], ht[:, k, 1:TB + 1],
                    ALU.mult, ALU.add)
                nc.tensor.matmul(aps1[:], c_w1[:, k, 0:128], xxx[:],
                                 start=(k == 0), stop=(k == KC - 1))
                nc.tensor.matmul(aps2[:], c_w1[:, k, 128:160], xxx[:],
                                 start=(k == 0), stop=(k == KC - 1))
            aTs = [cpool.tile([32, TB], BF16, tag=f"aT{i}", name="aTs")
                   for i in range(5)]
            for i in range(4):
                nc.scalar.activation(aTs[i][:], aps1[32 * i:32 * (i + 1), :],
                                     AF.Tanh)
            nc.scalar.activation(aTs[4][:], aps2[0:32, :], AF.Tanh)

            def a_slice(i):
                return aTs[i][:]

            def make_mix(i, tag):
                mt = big.tile([128, KC, TB], BF16, tag=tag, name="mixbuf")
                for k4 in range(4):
                    w2q = wstr.tile([32, 4, 128], BF16, tag="w2s", bufs=1)
                    bid = i * 4 + k4
                    nc.sync.dma_start(
                        w2q[:], maa_w2s[bid * 32:(bid + 1) * 32, :]
                        .rearrange("p (q f) -> p q f", q=4))
                    for q in range(4):
                        k = k4 * 4 + q
                        mp = pp(128, TB)
                        nc.tensor.matmul(mp[:], w2q[:, q, :], a_slice(i),
                                         start=True, stop=True)
                        t = sc.tile([128, TB], F32, tag="g1")
                        nc.vector.scalar_tensor_tensor(
                            t[:], mp[:], c_tm[:, k, i + 1:i + 2], xx[:, k, :],
                            ALU.add, ALU.mult)
                        nc.vector.tensor_add(mt[:, k, :], t[:],
                                             ht[:, k, 1:TB + 1])
                return mt

            def proj_cm(wp_ap, sink, src_view):
                for m in range(KC):
                    wt = wstr.tile([128, KC, 128], BF16, tag="wstream", bufs=3)
                    nc.sync.dma_start(
                        wt[:], wp_ap[m * 128:(m + 1) * 128, :]
                        .rearrange("p (k f) -> p k f", k=KC))
                    pt = pp(128, TB)
                    for k in range(KC):
                        nc.tensor.matmul(pt[:], wt[:, k, :], src_view(k),
                                         start=(k == 0), stop=(k == KC - 1))
                    sink(m, pt)

            def sink_a2a(idx):
                def s(m, pt):
                    st = sc.tile([128, TB], BF16, tag="g2")
                    nc.vector.tensor_copy(st[:], pt[:])
                    nc.sync.dma_start(
                        a2ark_in[m // 2, idx,
                                 128 * (m % 2):128 * (m % 2) + 128, :], st[:])
                return s

            # w decay first (cheap; its collective ships earliest)
            xw_t = make_mix(0, "mixt")
            t1p = pp(64, TB)
            for k in range(KC):
                nc.tensor.matmul(t1p[:], c_td1[:, k, :], xw_t[:, k, :],
                                 start=(k == 0), stop=(k == KC - 1))
            t1 = cpool.tile([64, TB], BF16, tag="t1")
            nc.scalar.activation(t1[:], t1p[:], AF.Tanh)
            nc.sync.dma_start(agt_in[:], t1[:])
            nc.gpsimd.collective_compute(
                "AllGather", ALU.bypass, replica_groups=RG,
                ins=[agt_in[:]], outs=[agt_out[:]])

            xr_t = make_mix(3, "mixt")
            proj_cm(Wp["r"], sink_a2a(0), lambda k: xr_t[:, k, :])
            xk_t = make_mix(1, "kfB")
            proj_cm(Wp["k"], sink_a2a(1), lambda k: xk_t[:, k, :])
            nc.gpsimd.collective_compute(
                "AllToAll", ALU.bypass, replica_groups=RG,
                ins=[a2ark_in[:]], outs=[a2ark_out[:]])

            # v (token-major out)
            xv_t = make_mix(2, "kfB")
            for cc in range(4):
                pvs = [pp(128, TB) for _ in range(4)]
                for k4 in range(4):
                    wv4 = wstr.tile([128, 4, TB], BF16, tag="wv_s", bufs=2)
                    nc.sync.dma_start(
                        wv4[:], Wv_p[(cc * 4 + k4) * 128:
                                     (cc * 4 + k4 + 1) * 128, :]
                        .rearrange("p (q f) -> p q f", q=4))
                    for q in range(4):
                        k = k4 * 4 + q
                        for t4 in range(4):
                            nc.tensor.matmul(
                                pvs[t4][:],
                                xv_t[:, k, 128 * t4:128 * (t4 + 1)],
                                wv4[:, q, :], start=(k == 0),
                                stop=(k == KC - 1))
                for t4 in range(4):
                    st = sc.tile([128, TB], BF16, tag="g2")
                    nc.vector.tensor_copy(st[:], pvs[t4][:])
                    for half in range(2):
                        nc.sync.dma_start(
                            a2v_in[2 * cc + half, 128 * t4:128 * (t4 + 1), :],
                            st[:, 256 * half:256 * (half + 1)])

            nc.gpsimd.collective_compute(
                "AllToAll", ALU.bypass, replica_groups=RG,
                ins=[a2v_in[:]], outs=[a2v_out[:]])

            # g projection raw (silu deferred; overlaps collectives / WKV)
            xg_t = make_mix(4, "mixt")

            def sink_g(m, pt):
                # gsb <- silu(g) = g/(1+exp(-g)), exp-table only
                sgf = sc.tile([128, TB], F32, tag="g1")
                nc.scalar.activation(sgf[:], pt[:], AF.Exp, scale=-1.0)
                nc.vector.tensor_scalar(sgf[:], sgf[:], 1.0, None, ALU.add)
                nc.vector.reciprocal(sgf[:], sgf[:])
                nc.vector.tensor_mul(gsb[:, m, :], pt[:], sgf[:])
            proj_cm(Wp["g"], sink_g, lambda k: xg_t[:, k, :])

            # ============ WKV (channel-major y, batched GN) ============
            for hp in range(2):
                for b in range(2):
                    S2b = cpool.tile([128, 64], BF16, tag=f"Sb_{hp}_{b}")
                    nc.vector.memset(S2b[:], 0.0)
                    for jb in range(4):
                        j = 4 * b + jb
                        hs = slice(128 * hp, 128 * (hp + 1))
                        r2 = scw.tile([128, TB], BF16, tag="wkv_r", bufs=2)
                        k2 = scw.tile([128, TB], BF16, tag="wkv_k", bufs=2)
                        t1j = scw.tile([64, TB], BF16, tag="wkv_w",
                                       bufs=2)
                        v2 = scw.tile([128, 4, 128], BF16, tag="wkv_v",
                                      bufs=2)
                        nc.sync.dma_start(r2[:], a2ark_out[j, 0, hs, :])
                        nc.sync.dma_start(k2[:], a2ark_out[j, 1, hs, :])
                        nc.sync.dma_start(t1j[:], agt_out[j, :, :])
                        nc.sync.dma_start(
                            v2[:], a2v_out[j, :, hs]
                            .rearrange("(cc p) c -> p cc c", p=128))
                        wps = pp(128, TB)
                        nc.tensor.matmul(wps[:], c_td2l[:, hp, :], t1j[:],
                                         start=True, stop=True)
                        e = scw.tile([128, TB], F32, tag="wkv_e",
                                     name="e")
                        nc.scalar.activation(e[:], wps[:], AF.Exp,
                                             bias=c_tdl[:, hp:hp + 1])
                        qe = scw.tile([128, TB], F32, tag="wkv_qe")
                        for cc in range(4):
                            cs = slice(128 * cc, 128 * (cc + 1))
                            nc.vector.tensor_tensor_scan(
                                qe[:, cs], e[:, cs], e[:, cs], 0.0,
                                ALU.add, ALU.bypass)
                        ku = scw.tile([128, TB], BF16, tag="wkv_ku")
                        nc.vector.tensor_scalar(ku[:], k2[:],
                                                c_u[:, hp:hp + 1], None,
                                                ALU.mult)
                        e2f = scw.tile([128, TB], BF16, tag="wkv_e2f")
                        nc.vector.tensor_mul(e2f[:], r2[:], ku[:])
                        # rt = r*exp(e-qe)
                        nc.vector.tensor_sub(e[:], e[:], qe[:])
                        nc.scalar.activation(e[:], e[:], AF.Exp)
                        rt = scw.tile([128, TB], BF16, tag="wkv_rt")
                        nc.vector.tensor_mul(rt[:], r2[:], e[:])
                        # kt = k*exp(qe)
                        ktt = scw.tile([128, TB], F32, tag="wkv_e",
                                       name="ktt")
                        nc.scalar.activation(ktt[:], qe[:], AF.Exp)
                        kt = scw.tile([128, TB], BF16, tag="wkv_kt")
                        nc.vector.tensor_mul(kt[:], k2[:], ktt[:])
                        ysb = scw.tile([128, TB], BF16, tag="wkv_ysb")
                        for cc in range(4):
                            cs = slice(128 * cc, 128 * (cc + 1))
                            qend = qe[:, 128 * cc + 127:128 * cc + 128]
                            pl2 = sc.tile([128, 1], F32, tag="wkv_pl")
                            nc.scalar.activation(pl2[:], qend, AF.Exp,
                                                 scale=-1.0)
                            kh = sc.tile([128, 128], BF16, tag="wkv_kh")
                            nc.vector.tensor_scalar(kh[:], kt[:, cs], pl2[:],
                                                    None, ALU.mult)
                            khT = ppb(128, 128)
                            nc.tensor.transpose(khT[:], kh[:], c_idb[:])
                            khTs = sc.tile([128, 128], BF16, tag="wkv_khTs")
                            nc.scalar.activation(khTs[:], khT[:], AF.Copy)
                            ypT = pp(128, 128)
                            sps = pp(128, 64)
                            for hh in range(2):
                                h64 = slice(64 * hh, 64 * (hh + 1))
                                at = pp(128, 128)
                                nc.tensor.matmul(at[:], kt[h64, cs],
                                                 rt[h64, cs],
                                                 start=True, stop=True)
                                scol = pp(128, 1)
                                nc.tensor.matmul(scol[:], e2f[h64, cs],
                                                 ones_colb[h64, :],
                                                 start=True, stop=True)
                                am = sc.tile([128, 128], BF16, tag="wkv_am")
                                nc.vector.tensor_mul(am[:], at[:], c_msk[:])
                                nc.vector.scalar_tensor_tensor(
                                    am[:], c_idb[:], scol[:], am[:],
                                    ALU.mult, ALU.add)
                                nc.tensor.matmul(ypT[h64, :], v2[:, cc, h64],
                                                 am[:], start=True, stop=False)
                                nc.tensor.matmul(ypT[h64, :], S2b[h64, :],
                                                 rt[h64, cs],
                                                 start=False, stop=True)
                                nc.tensor.matmul(sps[h64, :], khTs[:, h64],
                                                 v2[:, cc, h64],
                                                 start=True, stop=True)
                            nc.vector.scalar_tensor_tensor(
                                S2b[:], S2b[:], pl2[:], sps[:],
                                ALU.mult, ALU.add)
                            nc.scalar.activation(ysb[:, cs], ypT[:], AF.Copy)
                        # ---- batched GroupNorm over [128ch, TB] ----
                        ysq = sc.tile([128, TB], BF16, tag="wkv_ysq", bufs=1)
                        nc.scalar.activation(ysq[:], ysb[:], AF.Square)
                        sm = pp(2, TB)
                        nc.tensor.matmul(sm[:], c_blk2[:], ysb[:],
                                         start=True, stop=True)
                        sq2 = pp(2, TB)
                        nc.tensor.matmul(sq2[:], c_blk2[:], ysq[:],
                                         start=True, stop=True)
                        st2 = sc.tile([2, 2 * TB], F32, tag="wkv_st2", bufs=1)
                        mean2, inv2 = st2[:, 0:TB], st2[:, TB:2 * TB]
                        nc.scalar.activation(mean2[:], sm[:], AF.Copy,
                                             scale=1.0 / 64)
                        nc.scalar.activation(inv2[:], sq2[:], AF.Copy,
                                             scale=1.0 / 64)
                        # var = msq - mean^2 (in place); inv = (var+eps)^-1/2
                        nc.vector.tensor_mul(mean2[:], mean2[:], mean2[:])
                        nc.vector.tensor_sub(inv2[:], inv2[:], mean2[:])
                        nc.scalar.activation(mean2[:], sm[:], AF.Copy,
                                             scale=1.0 / 64)
                        nc.scalar.activation(inv2[:], inv2[:], AF.Ln,
                                             bias=EPS_LNX)
                        nc.scalar.activation(inv2[:], inv2[:], AF.Exp,
                                             scale=-0.5)
                        ab = sc.tile([2, 2 * TB], BF16, tag="wkv_ab", bufs=1)
                        nc.vector.tensor_copy(ab[:, 0:TB], inv2[:])
                        nc.vector.tensor_mul(mean2[:], mean2[:], inv2[:])
                        nc.vector.tensor_scalar(ab[:, TB:], mean2[:], -1.0,
                                                None, ALU.mult)
                        bca = pp(128, TB)
                        nc.tensor.matmul(bca[:], c_sel2[:], ab[:, 0:TB],
                                         start=True, stop=True)
                        bcb = pp(128, TB)
                        nc.tensor.matmul(bcb[:], c_sel2[:], ab[:, TB:],
                                         start=True, stop=True)
                        gn1 = sc.tile([128, TB], F32, tag="wkv_gn1", bufs=1)
                        nc.vector.tensor_mul(gn1[:], ysb[:], bca[:])
                        nc.vector.tensor_add(gn1[:], gn1[:], bcb[:])
                        ypb = sc.tile([128, TB], BF16, tag="wkv_ypb", bufs=1)
                        nc.vector.tensor_scalar(
                            ypb[:], gn1[:], c_lnxc[:, hp:hp + 1],
                            c_lnxc[:, 2 + hp:3 + hp], ALU.mult, ALU.add)
                        nc.sync.dma_start(a2b_in[j, hs, :], ypb[:])

            # ============ A2A back ============
            nc.gpsimd.collective_compute(
                "AllToAll", ALU.bypass, replica_groups=RG,
                ins=[a2b_in[:]], outs=[a2b_out[:]])

            # ============ att + residual ============
            yat = big.tile([128, KC, TB], BF16, tag="mixt", name="yat")
            for m in range(KC):
                yt = sc.tile([128, TB], BF16, tag="g1")
                nc.sync.dma_start(
                    yt[:],
                    a2b_out[m // 2, 128 * (m % 2):128 * (m % 2) + 128, :])
                nc.vector.tensor_mul(yat[:, m, :], yt[:], gsb[:, m, :])

            psA3 = ps.tile([1, TB], F32, tag="lnA", bufs=1)
            psA4 = ps.tile([1, TB], F32, tag="lnB", bufs=1)
            for m in range(KC):
                wt = wstr.tile([128, KC, 128], BF16, tag="wstream", bufs=3)
                nc.sync.dma_start(
                    wt[:], Wp["o"][m * 128:(m + 1) * 128, :]
                    .rearrange("p (k f) -> p k f", k=KC))
                pt = pp(128, TB)
                for k in range(KC):
                    nc.tensor.matmul(pt[:], wt[:, k, :], yat[:, k, :],
                                     start=(k == 0), stop=(k == KC - 1))
                x2t = sc.tile([128, TB], F32, tag="g3", bufs=1)
                xin = sc.tile([128, TB], F32, tag="g4", bufs=1)
                nc.sync.dma_start(xin[:], xT[128 * m:128 * (m + 1), 1:TB + 1])
                nc.vector.tensor_add(x2t[:], pt[:], xin[:])
                nc.sync.dma_start(x2d[128 * m:128 * (m + 1), 1:TB + 1], x2t[:])
                # ln2 stats accumulated inline (saves a full reload pass)
                sqx = sc.tile([128, TB], F32, tag="e2")
                nc.scalar.activation(sqx[:], x2t[:], AF.Square)
                st_, sp_ = (m == 0), (m == KC - 1)
                nc.tensor.matmul(psA3[:], ones_col[:], x2t[:],
                                 start=st_, stop=sp_)
                nc.tensor.matmul(psA4[:], ones_col[:], sqx[:],
                                 start=st_, stop=sp_)

            # ---- ln2 normalize pass (stats already accumulated) ----
            def x2_src(k):
                t = sc.tile([128, TB], F32, tag="e1")
                nc.sync.dma_start(t[:], x2d[128 * k:128 * (k + 1), 1:TB + 1])
                return t

            stats2 = lnp.tile([1, 2 * TB], F32, tag="ln_stats")
            mean2, msq2 = stats2[:, 0:TB], stats2[:, TB:]
            nc.scalar.activation(mean2[:], psA3[:], AF.Copy, scale=1.0 / C)
            nc.scalar.activation(msq2[:], psA4[:], AF.Copy, scale=1.0 / C)
            wk2 = lnp.tile([1, TB], F32, tag="ln_work")
            nc.vector.tensor_mul(wk2[:], mean2[:], mean2[:])
            nc.vector.tensor_sub(wk2[:], msq2[:], wk2[:])
            nc.scalar.activation(wk2[:], wk2[:], AF.Ln, bias=EPS_LN)
            nc.scalar.activation(wk2[:], wk2[:], AF.Exp, scale=-0.5)
            bmp3, bip3 = pp(128, TB), pp(128, TB)
            nc.tensor.matmul(bmp3[:], ones_row[:], mean2[:], start=True,
                             stop=True)
            nc.tensor.matmul(bip3[:], ones_row[:], wk2[:], start=True,
                             stop=True)
            bc2 = lnp.tile([128, 2 * TB], BF16, tag="ln_bc")
            nc.vector.tensor_copy(bc2[:, 0:TB], bmp3[:])
            nc.vector.tensor_copy(bc2[:, TB:], bip3[:])
            for k in range(KC):
                s = x2_src(k)
                t = sc.tile([128, TB], F32, tag="e2")
                nc.vector.tensor_sub(t[:], s[:], bc2[:, 0:TB])
                nc.vector.tensor_mul(t[:], t[:], bc2[:, TB:])
                nc.vector.tensor_scalar(ht[:, k, 1:TB + 1], t[:],
                                        c_ln2[:, k, 0:1], c_ln2[:, k, 1:2],
                                        ALU.mult, ALU.add)
                # h2 boundary (own last token) -> ag_in for the neighbor
                nc.sync.dma_start(ag_in[0:1, 128 * k:128 * (k + 1)],
                                  ht[:, k, TB:TB + 1])

            nc.gpsimd.collective_compute(
                "AllGather", ALU.bypass, replica_groups=RG,
                ins=[ag_in[:]], outs=[ag_out[:]])

            for q in range(4):
                agp = sc.tile([NCORE, TB], BF16, tag="agp", bufs=1)
                nc.sync.dma_start(agp[:], ag_out[:, 512 * q:512 * (q + 1)])
                hp_ = pp(1, TB)
                nc.tensor.matmul(hp_[:], c_sel[:], agp[:],
                                 start=True, stop=True)
                hrow = sc.tile([1, TB], BF16, tag="hrow")
                nc.vector.tensor_copy(hrow[:], hp_[:])
                for mm in range(4):
                    m = 4 * q + mm
                    nc.sync.dma_start(ht[:, m, 0:1],
                                      hrow[0:1, 128 * mm:128 * (mm + 1)])

            xk2b = big.tile([128, KC, TB], BF16, tag="xx")  # alias xx slot
            for k in range(KC):
                xx2 = sc.tile([128, TB], F32, tag="g1")
                nc.vector.tensor_sub(xx2[:], ht[:, k, 0:TB],
                                     ht[:, k, 1:TB + 1])
                nc.vector.scalar_tensor_tensor(
                    xk2b[:, k, :], xx2[:], c_cm[:, k, 0:1],
                    ht[:, k, 1:TB + 1], ALU.mult, ALU.add)    # xk2
                nc.vector.scalar_tensor_tensor(
                    gsb[:, k, :], xx2[:], c_cm[:, k, 1:2], ht[:, k, 1:TB + 1],
                    ALU.mult, ALU.add)        # xr2

            # ============ FFN ============
            kfA = big.tile([128, KC, TB], BF16, tag="mixt")  # alias mixt slot
            kfB = big.tile([128, 32, TB], BF16, tag="kfB")
            kfC = big.tile([128, 8, TB], BF16, tag="ht")     # alias ht slot

            def kf_view(i):
                if i < KC:
                    return kfA[:, i, :]
                return kfB[:, i - KC, :] if i < 48 else kfC[:, i - 48, :]

            for mf in range(KF):
                wt = wstr.tile([128, KC, 128], BF16, tag="wstream", bufs=3)
                nc.sync.dma_start(
                    wt[:], Wck_p[mf * 128:(mf + 1) * 128, :]
                    .rearrange("p (k f) -> p k f", k=KC))
                pt = pp(128, TB)
                for k in range(KC):
                    nc.tensor.matmul(pt[:], wt[:, k, :], xk2b[:, k, :],
                                     start=(k == 0), stop=(k == KC - 1))
                rl = sc.tile([128, TB], F32, tag="g1")
                nc.vector.tensor_scalar(rl[:], pt[:], 0.0, None, ALU.max)
                nc.scalar.activation(kf_view(mf), rl[:], AF.Square)

            for m in range(KC):
                ptu = pp(128, TB)
                for q in range(4):
                    wcv = wstr.tile([128, 14, 128], BF16, tag="wcv_s", bufs=2)
                    nc.sync.dma_start(
                        wcv[:], Wcv_p[m * 128:(m + 1) * 128,
                                      q * 14 * 128:(q + 1) * 14 * 128]
                        .rearrange("p (k f) -> p k f", k=14))
                    for kk in range(14):
                        ki = q * 14 + kk
                        nc.tensor.matmul(ptu[:], wcv[:, kk, :], kf_view(ki),
                                         start=(ki == 0), stop=(ki == KF - 1))
                wt = wstr.tile([128, KC, 128], BF16, tag="wstream", bufs=3)
                nc.sync.dma_start(
                    wt[:], Wp["cr"][m * 128:(m + 1) * 128, :]
                    .rearrange("p (k f) -> p k f", k=KC))
                pts = pp(128, TB)
                for k in range(KC):
                    nc.tensor.matmul(pts[:], wt[:, k, :], gsb[:, k, :],
                                     start=(k == 0), stop=(k == KC - 1))
                # sigmoid(x) = 1/(1+exp(-x)) via exp-table
                ssb = sc.tile([128, TB], F32, tag="g2")
                nc.scalar.activation(ssb[:], pts[:], AF.Exp, scale=-1.0)
                nc.vector.tensor_scalar(ssb[:], ssb[:], 1.0, None, ALU.add)
                nc.vector.reciprocal(ssb[:], ssb[:])
                ot = sc.tile([128, TB], F32, tag="g3", bufs=1)
                nc.vector.tensor_mul(ot[:], ptu[:], ssb[:])
                x2in = sc.tile([128, TB], F32, tag="g4", bufs=1)
                nc.sync.dma_start(x2in[:],
                                  x2d[128 * m:128 * (m + 1), 1:TB + 1])
                nc.vector.tensor_add(ot[:], ot[:], x2in[:])
                nc.sync.dma_start(outT[128 * m:128 * (m + 1), :], ot[:])

    nc.compile()
    return nc


_CACHE = {}


def _get_program():
    if "nc" not in _CACHE:
        _CACHE["nc"] = build_program()
    return _CACHE["nc"]


def _pret2(w):
    """Host pre-tile: out[m*128+p, k*128+f] = w[k*128+p, m*128+f]."""
    ci, co = w.shape
    KI, KO = ci // 128, co // 128
    return np.ascontiguousarray(
        w.reshape(KI, 128, KO, 128).transpose(2, 1, 0, 3)
    ).reshape(KO * 128, KI * 128)


def _shard_inputs(inp):
    f32 = np.float32
    x = np.asarray(inp["x"], f32)
    bf = lambda a: np.asarray(a, f32).astype(NP_BF16)

    maa_w2 = np.asarray(inp["maa_w2"], f32)         # (5, 32, C)
    maa_w2s = np.ascontiguousarray(
        maa_w2.reshape(5, 32, 4, 4, 128).transpose(0, 2, 1, 3, 4)
    ).reshape(5 * 4 * 32, 4 * 128)
    td_w2 = np.asarray(inp["td_w2"], f32)           # (64, C)
    Wv = np.asarray(inp["Wv"], f32)
    Wv_p = np.ascontiguousarray(
        Wv.reshape(4, 4, 128, 4, 512).transpose(3, 0, 2, 1, 4)
    ).reshape(4 * 4 * 128, 4 * 512)
    blk2 = np.zeros((128, 2), f32)
    blk2[0:64, 0] = 1.0
    blk2[64:128, 1] = 1.0

    shared = {
        "ln1_wb": np.stack([inp["ln1_w"], inp["ln1_b"]], 1).astype(f32),
        "ln2_wb": np.stack([inp["ln2_w"], inp["ln2_b"]], 1).astype(f32),
        "tm_maaT": np.asarray(inp["tm_maa"], f32).T.copy(),
        "cm_maaT": np.asarray(inp["cm_maa"], f32).T.copy(),
        "td_col": np.asarray(inp["time_decay"], f32).reshape(C, 1),
        "ident": np.eye(128, dtype=f32),
        "ident_bf": np.eye(128, dtype=f32).astype(NP_BF16),
        "mask_su": np.triu(np.ones((128, 128), f32), 1),
        "blk2h": blk2.astype(NP_BF16),
        "sel2h": np.ascontiguousarray(blk2.T).astype(NP_BF16),
        "maa_w1": bf(inp["maa_w1"]),
        "maa_w2s": maa_w2s.astype(NP_BF16),
        "td_w1": bf(inp["td_w1"]),
        "Wr_p": bf(_pret2(np.asarray(inp["Wr"], f32))),
        "Wk_p": bf(_pret2(np.asarray(inp["Wk"], f32))),
        "Wg_p": bf(_pret2(np.asarray(inp["Wg"], f32))),
        "Wo_p": bf(_pret2(np.asarray(inp["Wo"], f32))),
        "Wcr_p": bf(_pret2(np.asarray(inp["Wcr"], f32))),
        "Wv_p": Wv_p.astype(NP_BF16),
        "Wck_p": bf(_pret2(np.asarray(inp["Wck"], f32))),
        "Wcv_p": bf(_pret2(np.asarray(inp["Wcv"], f32))),
    }
    u = np.asarray(inp["time_faaaa"], f32).reshape(C)
    lnx_w = np.asarray(inp["lnx_w"], f32)
    lnx_b = np.asarray(inp["lnx_b"], f32)

    in_maps = []
    for c in range(NCORE):
        b, blk = c // 4, c % 4
        ts = blk * TB
        xe = np.zeros((C, TB + 1), f32)
        xe[:, 1:] = x[b, ts:ts + TB].T
        if blk > 0:
            xe[:, 0] = x[b, ts - 1]
        ul = u[LCH * c:LCH * (c + 1)].reshape(2, 128).T.copy()
        base = LCH * c
        tdw2l = np.ascontiguousarray(td_w2[:, base:base + 256])
        tdl = np.ascontiguousarray(
            np.asarray(inp["time_decay"], f32)[base:base + 256]
            .reshape(2, 128).T)
        lnxc = np.stack([lnx_w[base:base + 128], lnx_w[base + 128:base + 256],
                         lnx_b[base:base + 128], lnx_b[base + 128:base + 256]],
                        1).astype(f32)
        sel = np.zeros((NCORE, 1), NP_BF16)
        if blk > 0:
            sel[c - 1, 0] = 1.0
        m = dict(shared)
        m.update({
            "xT": xe,
            "halo_mask": np.full((128, 1), 1.0 if blk > 0 else 0.0, f32),
            "sel_prev": sel,
            "u_loc": ul,
            "lnx_cols": lnxc,
            "td_w2l": tdw2l.astype(NP_BF16),
            "td_loc": tdl,
        })
        in_maps.append(m)
    return in_maps


def run(inputs, trace=False):
    nc = _get_program()
    in_maps = _shard_inputs(inputs)
    res = bass_utils.run_bass_kernel_spmd(
        nc, in_maps, core_ids=list(range(NCORE)), trace=trace)
    x = np.asarray(inputs["x"], np.float32)
    out = np.empty_like(x)
    for c in range(NCORE):
        b, blk = c // 4, c % 4
        out[b, blk * TB:(blk + 1) * TB, :] = np.asarray(
            res.results[c]["out"], np.float32).T
    return out, res.exec_time_ns


def kernel(**inputs):
    out, _ = run(inputs)
    return out


if __name__ == "__main__":
    build_program()
    print("build ok")


# revision 54
# speedup vs baseline: 1.2935x; 1.2935x over previous
"""RWKV6 block (nn_Block_14602888806424) on 8 Trainium2 NeuronCores.

Token-sharded (sequence-parallel): each core owns 512 tokens (B=2 x 4
blocks); matmuls/LNs/mixing are token-local in channel-major layout.
r/k/w/v are redistributed head-sharded via bf16 AllToAll around the
chunked (L=128) WKV linear-attention scan (4 heads/core); the WKV inner
loop produces channel-major y directly (no output transposes) and runs a
batched GroupNorm per 512-token block with rsqrt = exp(-0.5*ln(var+eps))
so the whole scan stays on one activation table. A second bf16 AllToAll
returns gn(y); silu/sigmoid are computed as x/(1+exp(-x)) to avoid
activation-table swaps. A small AllGather carries the 1-token boundary
halo for the second token-shift. Projections/FFN in bf16 with f32 PSUM
accumulation; all streamed weights host-pre-tiled so every DMA is
partition-contiguous (4KB/partition lines).
"""

import sys
import numpy as np

sys.path.insert(0, "/opt/trn_rl_repo")

import concourse.bass as bass
import concourse.bacc as bacc
import concourse.mybir as mybir
import concourse.tile as tile
from concourse import bass_utils

F32 = mybir.dt.float32
BF16 = mybir.dt.bfloat16
NP_BF16 = mybir.dt.np(BF16)
AF = mybir.ActivationFunctionType
ALU = mybir.AluOpType

B, T, C, H, N, FF = 2, 2048, 2048, 32, 64, 7168
D_MIX, D_DECAY = 32, 64
EPS_LN = 1e-5
EPS_LNX = 1e-5 * 8.0**2
NCORE = 8
TB = 512
KC = C // 128          # 16
KF = FF // 128         # 56
LCH = 256              # channels per core (4 heads)
RG = [list(range(NCORE))]


def build_program():
    nc = bacc.Bacc("TRN2", target_bir_lowering=False, debug=False,
                   num_devices=NCORE, enable_asserts=False)

    def din(name, shape, dt=F32):
        return nc.dram_tensor(name, list(shape), dt, kind="ExternalInput").ap()

    xT = din("xT", (C, TB + 1))
    halo_mask = din("halo_mask", (128, 1))
    sel_prev = din("sel_prev", (NCORE, 1), BF16)
    u_loc = din("u_loc", (128, 2))
    lnx_cols = din("lnx_cols", (128, 4))
    ln1_wb = din("ln1_wb", (C, 2))
    ln2_wb = din("ln2_wb", (C, 2))
    tm_maaT = din("tm_maaT", (C, 6))
    cm_maaT = din("cm_maaT", (C, 2))
    td_col = din("td_col", (C, 1))
    ident = din("ident", (128, 128))
    ident_bf = din("ident_bf", (128, 128), BF16)
    mask_su = din("mask_su", (128, 128))
    blk2h = din("blk2h", (128, 2), BF16)
    sel2w = din("sel2w", (2, 2 * 128), BF16)
    maa_w1 = din("maa_w1", (C, 5 * D_MIX), BF16)
    maa_w2s = din("maa_w2s", (5 * 4 * 32, 4 * 128), BF16)
    td_w1 = din("td_w1", (C, D_DECAY), BF16)
    td_w2f = din("td_w2f", (64, KC * 128), BF16)
    Wp = {k: din(f"W{k}_p", (C, KC * 128), BF16)
          for k in ["r", "k", "g", "o", "cr"]}
    Wv_p = din("Wv_p", (4 * 4 * 128, 4 * 512), BF16)
    Wck_p = din("Wck_p", (KF * 128, KC * 128), BF16)
    Wcv_p = din("Wcv_p", (KC * 128, KF * 128), BF16)

    outT = nc.dram_tensor("out", [C, TB], F32, kind="ExternalOutput").ap()

    with tile.TileContext(nc) as tc:
        import contextlib
        with contextlib.ExitStack() as ctx:
            dram = ctx.enter_context(tc.tile_pool(name="dram", bufs=1,
                                                  space="DRAM"))
            cpool = ctx.enter_context(tc.tile_pool(name="const", bufs=1))
            big = ctx.enter_context(tc.tile_pool(name="big", bufs=1))
            wstr = ctx.enter_context(tc.tile_pool(name="wstr", bufs=3))
            sc = ctx.enter_context(tc.tile_pool(name="scratch", bufs=2))
            scw = ctx.enter_context(tc.tile_pool(name="scw", bufs=1))
            lnp = ctx.enter_context(tc.tile_pool(name="lnp", bufs=1))
            ps = ctx.enter_context(
                tc.tile_pool(name="psum", bufs=6, space="PSUM"))

            def pp(p_, f_):
                return ps.tile([p_, f_], F32, tag="pp", name="pp")

            def ppb(p_, f_):
                return ps.tile([p_, f_], BF16, tag="pp", name="pp")

            # ---- DRAM internals (collective payloads in bf16) ----
            a2aw_in = dram.tile([NCORE, LCH, TB], BF16, tag="a2aw_in")
            a2aw_out = dram.tile([NCORE, LCH, TB], BF16, tag="a2aw_out")
            a2ark_in = dram.tile([NCORE, 2, LCH, TB], BF16, tag="a2ark_in")
            a2ark_out = dram.tile([NCORE, 2, LCH, TB], BF16,
                                  tag="a2ark_out")
            a2v_in = dram.tile([NCORE, TB, LCH], BF16, tag="a2v_in")
            a2v_out = dram.tile([NCORE, TB, LCH], BF16, tag="a2v_out")
            a2b_in = dram.tile([NCORE, LCH, TB], BF16, tag="a2b_in")
            a2b_out = dram.tile([NCORE, LCH, TB], BF16, tag="a2b_out")
            ag_in = dram.tile([1, C], BF16, tag="ag_in")
            ag_out = dram.tile([NCORE, C], BF16, tag="ag_out",
                               addr_space="Shared")
            x2d = dram.tile([C, TB + 1], F32, tag="x2d")

            # ---- constants ----
            def cload(name, src, shape, dt=F32, rearr=None, **kw):
                t = cpool.tile(list(shape), dt, tag=name)
                nc.sync.dma_start(t[:], src if rearr is None
                                  else src.rearrange(rearr, **kw))
                return t

            c_ln1 = cload("c_ln1", ln1_wb, (128, KC, 2), F32,
                          "(k p) f -> p k f", p=128)
            c_ln2 = cload("c_ln2", ln2_wb, (128, KC, 2), F32,
                          "(k p) f -> p k f", p=128)
            c_tm = cload("c_tm", tm_maaT, (128, KC, 6), F32,
                         "(k p) f -> p k f", p=128)
            c_cm = cload("c_cm", cm_maaT, (128, KC, 2), F32,
                         "(k p) f -> p k f", p=128)
            c_td = cload("c_td", td_col, (128, KC, 1), F32,
                         "(k p) f -> p k f", p=128)
            c_hm = cload("c_hm", halo_mask, (128, 1))
            c_sel = cload("c_sel", sel_prev, (NCORE, 1), BF16)
            c_u = cload("c_u", u_loc, (128, 2))
            c_lnxc = cload("c_lnxc", lnx_cols, (128, 4))
            c_idb = cload("c_idb", ident_bf, (128, 128), BF16)
            c_msk = cload("c_msk", mask_su, (128, 128))
            c_blk2 = cload("c_blk2", blk2h, (128, 2), BF16)
            c_sel2w = cload("c_sel2w", sel2w, (2, 2, 128), BF16,
                            "p (h f) -> p h f", h=2)
            c_w1 = cload("c_w1", maa_w1, (128, KC, 5 * D_MIX), BF16,
                         "(k p) f -> p k f", p=128)
            c_td1 = cload("c_td1", td_w1, (128, KC, D_DECAY), BF16,
                          "(k p) f -> p k f", p=128)
            c_td2 = cload("c_td2", td_w2f, (64, KC, 128), BF16,
                          "p (k f) -> p k f", k=KC)
            ones_col = cpool.tile([128, 1], F32, tag="ones_col")
            nc.vector.memset(ones_col[:], 1.0)
            ones_colb = cpool.tile([128, 1], BF16, tag="ones_colb")
            nc.vector.memset(ones_colb[:], 1.0)
            ones_row = cpool.tile([1, 128], F32, tag="ones_row")
            nc.vector.memset(ones_row[:], 1.0)
            for _cv in (EPS_LN, EPS_LNX):
                cvt = cpool.tile([128, 1], F32, tag=f"cv{_cv}", name="cvt")
                nc.vector.memset(cvt[:], _cv)
                nc.const_aps.aps[(F32, _cv)] = cvt[:]

            # ---- persistent SBUF ----
            ht = big.tile([128, KC, TB + 1], BF16, tag="ht")
            xx = big.tile([128, KC, TB], BF16, tag="xx")      # later xk2
            gsb = big.tile([128, KC, TB], BF16, tag="gsb")    # later xr2

            # ============ layernorm over TB+1 cols ============
            def layer_norm_ext(src_fn, dst_view, wb, eps):
                """src_fn(k)->(128,TB+1) f32 AP-producing fn; called twice."""
                psA, psB = pp(1, TB), pp(1, 1)
                psA2, psB2 = pp(1, TB), pp(1, 1)
                for k in range(KC):
                    s = src_fn(k)
                    sq = sc.tile([128, TB + 1], F32, tag="e2")
                    nc.scalar.activation(sq[:], s[:], AF.Square)
                    st, sp = (k == 0), (k == KC - 1)
                    nc.tensor.matmul(psA[:], ones_col[:], s[:, 0:TB],
                                     start=st, stop=sp)
                    nc.tensor.matmul(psB[:], ones_col[:], s[:, TB:TB + 1],
                                     start=st, stop=sp)
                    nc.tensor.matmul(psA2[:], ones_col[:], sq[:, 0:TB],
                                     start=st, stop=sp)
                    nc.tensor.matmul(psB2[:], ones_col[:], sq[:, TB:TB + 1],
                                     start=st, stop=sp)
                stats = lnp.tile([1, 2 * (TB + 1)], F32, tag="ln_stats")
                mean, msq = stats[:, 0:TB + 1], stats[:, TB + 1:]
                nc.scalar.activation(mean[:, 0:TB], psA[:], AF.Copy,
                                     scale=1.0 / C)
                nc.scalar.activation(mean[:, TB:TB + 1], psB[:], AF.Copy,
                                     scale=1.0 / C)
                nc.scalar.activation(msq[:, 0:TB], psA2[:], AF.Copy,
                                     scale=1.0 / C)
                nc.scalar.activation(msq[:, TB:TB + 1], psB2[:], AF.Copy,
                                     scale=1.0 / C)
                wk = lnp.tile([1, TB + 1], F32, tag="ln_work")
                nc.vector.tensor_mul(wk[:], mean[:], mean[:])
                nc.vector.tensor_sub(wk[:], msq[:], wk[:])
                nc.scalar.activation(wk[:], wk[:], AF.Ln, bias=eps)
                nc.scalar.activation(wk[:], wk[:], AF.Exp, scale=-0.5)
                bmp, bmp2 = pp(128, TB), pp(128, 1)
                bip, bip2 = pp(128, TB), pp(128, 1)
                nc.tensor.matmul(bmp[:], ones_row[:], mean[:, 0:TB],
                                 start=True, stop=True)
                nc.tensor.matmul(bmp2[:], ones_row[:], mean[:, TB:TB + 1],
                                 start=True, stop=True)
                nc.tensor.matmul(bip[:], ones_row[:], wk[:, 0:TB],
                                 start=True, stop=True)
                nc.tensor.matmul(bip2[:], ones_row[:], wk[:, TB:TB + 1],
                                 start=True, stop=True)
                bc = lnp.tile([128, 2 * (TB + 1)], BF16, tag="ln_bc")
                bm, bi = bc[:, 0:TB + 1], bc[:, TB + 1:]
                nc.vector.tensor_copy(bm[:, 0:TB], bmp[:])
                nc.vector.tensor_copy(bm[:, TB:TB + 1], bmp2[:])
                nc.vector.tensor_copy(bi[:, 0:TB], bip[:])
                nc.vector.tensor_copy(bi[:, TB:TB + 1], bip2[:])
                for k in range(KC):
                    s = src_fn(k)
                    t = sc.tile([128, TB + 1], F32, tag="e2")
                    nc.vector.tensor_sub(t[:], s[:], bm[:])
                    nc.vector.tensor_mul(t[:], t[:], bi[:])
                    d = dst_view(k)
                    nc.vector.tensor_scalar(d, t[:], wb[:, k, 0:1],
                                            wb[:, k, 1:2], ALU.mult, ALU.add)
                    nc.vector.tensor_scalar(d[:, 0:1], d[:, 0:1], c_hm[:],
                                            None, ALU.mult)

            def xt_src(k):
                t = sc.tile([128, TB + 1], F32, tag="e1", bufs=2)
                nc.sync.dma_start(t[:], xT[128 * k:128 * (k + 1), :])
                return t

            # ============ phase A: ln1 + shift ============
            layer_norm_ext(xt_src, lambda k: ht[:, k, :], c_ln1, EPS_LN)
            for k in range(KC):
                nc.vector.tensor_sub(xx[:, k, :], ht[:, k, 0:TB],
                                     ht[:, k, 1:TB + 1])

            # ============ phase B: maa ============
            aps1, aps2 = pp(128, TB), pp(32, TB)
            for k in range(KC):
                xxx = sc.tile([128, TB], BF16, tag="xxx")
                nc.vector.scalar_tensor_tensor(
                    xxx[:], xx[:, k, :], c_tm[:, k, 0:1], ht[:, k, 1:TB + 1],
                    ALU.mult, ALU.add)
                nc.tensor.matmul(aps1[:], c_w1[:, k, 0:128], xxx[:],
                                 start=(k == 0), stop=(k == KC - 1))
                nc.tensor.matmul(aps2[:], c_w1[:, k, 128:160], xxx[:],
                                 start=(k == 0), stop=(k == KC - 1))
            aTs = [cpool.tile([32, TB], BF16, tag=f"aT{i}", name="aTs")
                   for i in range(5)]
            for i in range(4):
                nc.scalar.activation(aTs[i][:], aps1[32 * i:32 * (i + 1), :],
                                     AF.Tanh)
            nc.scalar.activation(aTs[4][:], aps2[0:32, :], AF.Tanh)

            def a_slice(i):
                return aTs[i][:]

            def make_mix(i, tag):
                mt = big.tile([128, KC, TB], BF16, tag=tag, name="mixbuf")
                for k4 in range(4):
                    w2q = wstr.tile([32, 4, 128], BF16, tag="w2s", bufs=1)
                    bid = i * 4 + k4
                    nc.sync.dma_start(
                        w2q[:], maa_w2s[bid * 32:(bid + 1) * 32, :]
                        .rearrange("p (q f) -> p q f", q=4))
                    for q in range(4):
                        k = k4 * 4 + q
                        mp = pp(128, TB)
                        nc.tensor.matmul(mp[:], w2q[:, q, :], a_slice(i),
                                         start=True, stop=True)
                        t = sc.tile([128, TB], F32, tag="g1")
                        nc.vector.scalar_tensor_tensor(
                            t[:], mp[:], c_tm[:, k, i + 1:i + 2], xx[:, k, :],
                            ALU.add, ALU.mult)
                        nc.vector.tensor_add(mt[:, k, :], t[:],
                                             ht[:, k, 1:TB + 1])
                return mt

            def proj_cm(wp_ap, sink, src_view):
                for m in range(KC):
                    wt = wstr.tile([128, KC, 128], BF16, tag="wstream", bufs=3)
                    nc.sync.dma_start(
                        wt[:], wp_ap[m * 128:(m + 1) * 128, :]
                        .rearrange("p (k f) -> p k f", k=KC))
                    pt = pp(128, TB)
                    for k in range(KC):
                        nc.tensor.matmul(pt[:], wt[:, k, :], src_view(k),
                                         start=(k == 0), stop=(k == KC - 1))
                    sink(m, pt)

            def sink_a2a(idx):
                def s(m, pt):
                    st = sc.tile([128, TB], BF16, tag="g2")
                    nc.vector.tensor_copy(st[:], pt[:])
                    nc.sync.dma_start(
                        a2ark_in[m // 2, idx,
                                 128 * (m % 2):128 * (m % 2) + 128, :], st[:])
                return s

            # w decay first (cheap; its collective ships earliest)
            xw_t = make_mix(0, "mixt")
            t1p = pp(64, TB)
            for k in range(KC):
                nc.tensor.matmul(t1p[:], c_td1[:, k, :], xw_t[:, k, :],
                                 start=(k == 0), stop=(k == KC - 1))
            t1 = cpool.tile([64, TB], BF16, tag="t1")
            nc.scalar.activation(t1[:], t1p[:], AF.Tanh)
            for m in range(KC):
                wp2 = pp(128, TB)
                nc.tensor.matmul(wp2[:], c_td2[:, m, :], t1[:],
                                 start=True, stop=True)
                st = sc.tile([128, TB], BF16, tag="g2")
                nc.vector.tensor_scalar(st[:], wp2[:], c_td[:, m, 0:1], None,
                                        ALU.add)
                nc.sync.dma_start(
                    a2aw_in[m // 2, 128 * (m % 2):128 * (m % 2) + 128, :],
                    st[:])
            nc.gpsimd.collective_compute(
                "AllToAll", ALU.bypass, replica_groups=RG,
                ins=[a2aw_in[:]], outs=[a2aw_out[:]])

            xr_t = make_mix(3, "mixt")
            proj_cm(Wp["r"], sink_a2a(0), lambda k: xr_t[:, k, :])
            xk_t = make_mix(1, "kfB")
            proj_cm(Wp["k"], sink_a2a(1), lambda k: xk_t[:, k, :])
            nc.gpsimd.collective_compute(
                "AllToAll", ALU.bypass, replica_groups=RG,
                ins=[a2ark_in[:]], outs=[a2ark_out[:]])

            # v (token-major out)
            xv_t = make_mix(2, "kfB")
            for cc in range(4):
                pvs = [pp(128, TB) for _ in range(4)]
                for k4 in range(4):
                    wv4 = wstr.tile([128, 4, TB], BF16, tag="wv_s", bufs=2)
                    nc.sync.dma_start(
                        wv4[:], Wv_p[(cc * 4 + k4) * 128:
                                     (cc * 4 + k4 + 1) * 128, :]
                        .rearrange("p (q f) -> p q f", q=4))
                    for q in range(4):
                        k = k4 * 4 + q
                        for t4 in range(4):
                            nc.tensor.matmul(
                                pvs[t4][:],
                                xv_t[:, k, 128 * t4:128 * (t4 + 1)],
                                wv4[:, q, :], start=(k == 0),
                                stop=(k == KC - 1))
                for t4 in range(4):
                    st = sc.tile([128, TB], BF16, tag="g2")
                    nc.vector.tensor_copy(st[:], pvs[t4][:])
                    for half in range(2):
                        nc.sync.dma_start(
                            a2v_in[2 * cc + half, 128 * t4:128 * (t4 + 1), :],
                            st[:, 256 * half:256 * (half + 1)])

            nc.gpsimd.collective_compute(
                "AllToAll", ALU.bypass, replica_groups=RG,
                ins=[a2v_in[:]], outs=[a2v_out[:]])

            # g projection raw (silu deferred; overlaps collectives / WKV)
            xg_t = make_mix(4, "mixt")

            def sink_g(m, pt):
                # gsb <- silu(g) = g/(1+exp(-g)), exp-table only
                sgf = sc.tile([128, TB], F32, tag="g1")
                nc.scalar.activation(sgf[:], pt[:], AF.Exp, scale=-1.0)
                nc.vector.tensor_scalar(sgf[:], sgf[:], 1.0, None, ALU.add)
                nc.vector.reciprocal(sgf[:], sgf[:])
                nc.vector.tensor_mul(gsb[:, m, :], pt[:], sgf[:])
            proj_cm(Wp["g"], sink_g, lambda k: xg_t[:, k, :])

            # ============ WKV (channel-major y, batched GN) ============
            for hp in range(2):
                for b in range(2):
                    S2b = cpool.tile([128, 64], BF16, tag=f"Sb_{hp}_{b}")
                    nc.vector.memset(S2b[:], 0.0)
                    for jb in range(4):
                        j = 4 * b + jb
                        hs = slice(128 * hp, 128 * (hp + 1))
                        r2 = scw.tile([128, TB], BF16, tag="wkv_r", bufs=2)
                        k2 = scw.tile([128, TB], BF16, tag="wkv_k", bufs=2)
                        w2 = scw.tile([128, TB], BF16, tag="wkv_w", bufs=2)
                        v2 = scw.tile([128, 4, 128], BF16, tag="wkv_v",
                                      bufs=2)
                        nc.sync.dma_start(r2[:], a2ark_out[j, 0, hs, :])
                        nc.sync.dma_start(k2[:], a2ark_out[j, 1, hs, :])
                        nc.sync.dma_start(w2[:], a2aw_out[j, hs, :])
                        nc.sync.dma_start(
                            v2[:], a2v_out[j, :, hs]
                            .rearrange("(cc p) c -> p cc c", p=128))
                        e = scw.tile([128, TB], F32, tag="wkv_e",
                                     name="e")
                        nc.scalar.activation(e[:], w2[:], AF.Exp)
                        qe = scw.tile([128, TB], F32, tag="wkv_qe")
                        for cc in range(4):
                            cs = slice(128 * cc, 128 * (cc + 1))
                            nc.vector.tensor_tensor_scan(
                                qe[:, cs], e[:, cs], e[:, cs], 0.0,
                                ALU.add, ALU.bypass)
                        ku = scw.tile([128, TB], BF16, tag="wkv_ku")
                        nc.vector.tensor_scalar(ku[:], k2[:],
                                                c_u[:, hp:hp + 1], None,
                                                ALU.mult)
                        e2f = scw.tile([128, TB], BF16, tag="wkv_e2f")
                        nc.vector.tensor_mul(e2f[:], r2[:], ku[:])
                        # rt = r*exp(e-qe)
                        nc.vector.tensor_sub(e[:], e[:], qe[:])
                        nc.scalar.activation(e[:], e[:], AF.Exp)
                        rt = scw.tile([128, TB], BF16, tag="wkv_rt")
                        nc.vector.tensor_mul(rt[:], r2[:], e[:])
                        # kt = k*exp(qe)
                        ktt = scw.tile([128, TB], F32, tag="wkv_e",
                                       name="ktt")
                        nc.scalar.activation(ktt[:], qe[:], AF.Exp)
                        kt = scw.tile([128, TB], BF16, tag="wkv_kt")
                        nc.vector.tensor_mul(kt[:], k2[:], ktt[:])
                        ysb = scw.tile([128, TB], BF16, tag="wkv_ysb")
                        for cc in range(4):
                            cs = slice(128 * cc, 128 * (cc + 1))
                            qend = qe[:, 128 * cc + 127:128 * cc + 128]
                            pl2 = sc.tile([128, 1], F32, tag="wkv_pl")
                            nc.scalar.activation(pl2[:], qend, AF.Exp,
                                                 scale=-1.0)
                            kh = sc.tile([128, 128], BF16, tag="wkv_kh")
                            nc.vector.tensor_scalar(kh[:], kt[:, cs], pl2[:],
                                                    None, ALU.mult)
                            khT = ppb(128, 128)
                            nc.tensor.transpose(khT[:], kh[:], c_idb[:])
                            khTs = sc.tile([128, 128], BF16, tag="wkv_khTs")
                            nc.scalar.activation(khTs[:], khT[:], AF.Copy)
                            ypT = pp(128, 128)
                            sps = pp(128, 64)
                            for hh in range(2):
                                h64 = slice(64 * hh, 64 * (hh + 1))
                                at = pp(128, 128)
                                nc.tensor.matmul(at[:], kt[h64, cs],
                                                 rt[h64, cs],
                                                 start=True, stop=True)
                                scol = pp(128, 1)
                                nc.tensor.matmul(scol[:], e2f[h64, cs],
                                                 ones_colb[h64, :],
                                                 start=True, stop=True)
                                am = sc.tile([128, 128], BF16, tag="wkv_am")
                                nc.vector.tensor_mul(am[:], at[:], c_msk[:])
                                nc.vector.scalar_tensor_tensor(
                                    am[:], c_idb[:], scol[:], am[:],
                                    ALU.mult, ALU.add)
                                nc.tensor.matmul(ypT[h64, :], v2[:, cc, h64],
                                                 am[:], start=True, stop=False)
                                nc.tensor.matmul(ypT[h64, :], S2b[h64, :],
                                                 rt[h64, cs],
                                                 start=False, stop=True)
                                nc.tensor.matmul(sps[h64, :], khTs[:, h64],
                                                 v2[:, cc, h64],
                                                 start=True, stop=True)
                            nc.vector.scalar_tensor_tensor(
                                S2b[:], S2b[:], pl2[:], sps[:],
                                ALU.mult, ALU.add)
                            nc.scalar.activation(ysb[:, cs], ypT[:], AF.Copy)
                        # ---- batched GroupNorm over [128ch, TB] ----
                        ysq = sc.tile([128, TB], BF16, tag="wkv_ysq", bufs=1)
                        nc.scalar.activation(ysq[:], ysb[:], AF.Square)
                        sm = pp(2, TB)
                        nc.tensor.matmul(sm[:], c_blk2[:], ysb[:],
                                         start=True, stop=True)
                        sq2 = pp(2, TB)
                        nc.tensor.matmul(sq2[:], c_blk2[:], ysq[:],
                                         start=True, stop=True)
                        st2 = sc.tile([2, 2 * TB], F32, tag="wkv_st2", bufs=1)
                        mean2, inv2 = st2[:, 0:TB], st2[:, TB:2 * TB]
                        nc.scalar.activation(mean2[:], sm[:], AF.Copy,
                                             scale=1.0 / 64)
                        nc.scalar.activation(inv2[:], sq2[:], AF.Copy,
                                             scale=1.0 / 64)
                        # var = msq - mean^2 (in place); inv = (var+eps)^-1/2
                        nc.vector.tensor_mul(mean2[:], mean2[:], mean2[:])
                        nc.vector.tensor_sub(inv2[:], inv2[:], mean2[:])
                        nc.scalar.activation(mean2[:], sm[:], AF.Copy,
                                             scale=1.0 / 64)
                        nc.scalar.activation(inv2[:], inv2[:], AF.Ln,
                                             bias=EPS_LNX)
                        nc.scalar.activation(inv2[:], inv2[:], AF.Exp,
                                             scale=-0.5)
                        ab = sc.tile([2, 2 * TB], BF16, tag="wkv_ab", bufs=1)
                        nc.vector.tensor_copy(ab[:, 0:TB], inv2[:])
                        nc.vector.tensor_mul(mean2[:], mean2[:], inv2[:])
                        nc.vector.tensor_scalar(ab[:, TB:], mean2[:], -1.0,
                                                None, ALU.mult)
                        bca = pp(128, TB)
                        nc.tensor.matmul(bca[:], c_sel2w[:, hp, :],
                                         ab[:, 0:TB], start=True, stop=True)
                        bcb = pp(128, TB)
                        nc.tensor.matmul(bcb[:], c_sel2w[:, hp, :],
                                         ab[:, TB:], start=True, stop=True)
                        gn1 = sc.tile([128, TB], F32, tag="wkv_gn1", bufs=1)
                        nc.vector.tensor_mul(gn1[:], ysb[:], bca[:])
                        ypb = sc.tile([128, TB], BF16, tag="wkv_ypb", bufs=1)
                        nc.vector.scalar_tensor_tensor(
                            ypb[:], gn1[:], c_lnxc[:, 2 + hp:3 + hp], bcb[:],
                            ALU.add, ALU.add)
                        nc.sync.dma_start(a2b_in[j, hs, :], ypb[:])

            # ============ A2A back ============
            nc.gpsimd.collective_compute(
                "AllToAll", ALU.bypass, replica_groups=RG,
                ins=[a2b_in[:]], outs=[a2b_out[:]])

            # ============ att + residual ============
            yat = big.tile([128, KC, TB], BF16, tag="mixt", name="yat")
            for j in range(NCORE):
                ytp = sc.tile([128, 2, TB], BF16, tag="g1", name="ytp")
                nc.sync.dma_start(
                    ytp[:],
                    a2b_out[j].rearrange("(m2 p) t -> p m2 t", p=128))
                for m2 in range(2):
                    m = 2 * j + m2
                    nc.vector.tensor_mul(yat[:, m, :], ytp[:, m2, :],
                                         gsb[:, m, :])

            psA3 = ps.tile([1, TB], F32, tag="lnA", bufs=1)
            psA4 = ps.tile([1, TB], F32, tag="lnB", bufs=1)
            for m in range(KC):
                wt = wstr.tile([128, KC, 128], BF16, tag="wstream", bufs=3)
                nc.sync.dma_start(
                    wt[:], Wp["o"][m * 128:(m + 1) * 128, :]
                    .rearrange("p (k f) -> p k f", k=KC))
                pt = pp(128, TB)
                for k in range(KC):
                    nc.tensor.matmul(pt[:], wt[:, k, :], yat[:, k, :],
                                     start=(k == 0), stop=(k == KC - 1))
                x2t = sc.tile([128, TB], F32, tag="g3", bufs=1)
                xin = sc.tile([128, TB], F32, tag="g4", bufs=1)
                nc.sync.dma_start(xin[:], xT[128 * m:128 * (m + 1), 1:TB + 1])
                nc.vector.tensor_add(x2t[:], pt[:], xin[:])
                nc.sync.dma_start(x2d[128 * m:128 * (m + 1), 1:TB + 1], x2t[:])
                # ln2 stats accumulated inline (saves a full reload pass)
                sqx = sc.tile([128, TB], F32, tag="e2")
                nc.scalar.activation(sqx[:], x2t[:], AF.Square)
                st_, sp_ = (m == 0), (m == KC - 1)
                nc.tensor.matmul(psA3[:], ones_col[:], x2t[:],
                                 start=st_, stop=sp_)
                nc.tensor.matmul(psA4[:], ones_col[:], sqx[:],
                                 start=st_, stop=sp_)

            # ---- ln2 normalize pass (stats already accumulated) ----
            def x2_src(k):
                t = sc.tile([128, TB], F32, tag="e1")
                nc.sync.dma_start(t[:], x2d[128 * k:128 * (k + 1), 1:TB + 1])
                return t

            stats2 = lnp.tile([1, 2 * TB], F32, tag="ln_stats")
            mean2, msq2 = stats2[:, 0:TB], stats2[:, TB:]
            nc.scalar.activation(mean2[:], psA3[:], AF.Copy, scale=1.0 / C)
            nc.scalar.activation(msq2[:], psA4[:], AF.Copy, scale=1.0 / C)
            wk2 = lnp.tile([1, TB], F32, tag="ln_work")
            nc.vector.tensor_mul(wk2[:], mean2[:], mean2[:])
            nc.vector.tensor_sub(wk2[:], msq2[:], wk2[:])
            nc.scalar.activation(wk2[:], wk2[:], AF.Ln, bias=EPS_LN)
            nc.scalar.activation(wk2[:], wk2[:], AF.Exp, scale=-0.5)
            bmp3, bip3 = pp(128, TB), pp(128, TB)
            nc.tensor.matmul(bmp3[:], ones_row[:], mean2[:], start=True,
                             stop=True)
            nc.tensor.matmul(bip3[:], ones_row[:], wk2[:], start=True,
                             stop=True)
            bc2 = lnp.tile([128, 2 * TB], BF16, tag="ln_bc")
            nc.vector.tensor_copy(bc2[:, 0:TB], bmp3[:])
            nc.vector.tensor_copy(bc2[:, TB:], bip3[:])
            for k in range(KC):
                s = x2_src(k)
                t = sc.tile([128, TB], F32, tag="e2")
                nc.vector.tensor_sub(t[:], s[:], bc2[:, 0:TB])
                nc.vector.tensor_mul(t[:], t[:], bc2[:, TB:])
                nc.vector.tensor_scalar(ht[:, k, 1:TB + 1], t[:],
                                        c_ln2[:, k, 0:1], c_ln2[:, k, 1:2],
                                        ALU.mult, ALU.add)
                # h2 boundary (own last token) -> ag_in for the neighbor
                nc.sync.dma_start(ag_in[0:1, 128 * k:128 * (k + 1)],
                                  ht[:, k, TB:TB + 1])

            nc.gpsimd.collective_compute(
                "AllGather", ALU.bypass, replica_groups=RG,
                ins=[ag_in[:]], outs=[ag_out[:]])

            for q in range(4):
                agp = sc.tile([NCORE, TB], BF16, tag="agp", bufs=1)
                nc.sync.dma_start(agp[:], ag_out[:, 512 * q:512 * (q + 1)])
                hp_ = pp(1, TB)
                nc.tensor.matmul(hp_[:], c_sel[:], agp[:],
                                 start=True, stop=True)
                hrow = sc.tile([1, TB], BF16, tag="hrow")
                nc.vector.tensor_copy(hrow[:], hp_[:])
                for mm in range(4):
                    m = 4 * q + mm
                    nc.sync.dma_start(ht[:, m, 0:1],
                                      hrow[0:1, 128 * mm:128 * (mm + 1)])

            xk2b = big.tile([128, KC, TB], BF16, tag="xx")  # alias xx slot
            for k in range(KC):
                xx2 = sc.tile([128, TB], F32, tag="g1")
                nc.vector.tensor_sub(xx2[:], ht[:, k, 0:TB],
                                     ht[:, k, 1:TB + 1])
                nc.vector.scalar_tensor_tensor(
                    xk2b[:, k, :], xx2[:], c_cm[:, k, 0:1],
                    ht[:, k, 1:TB + 1], ALU.mult, ALU.add)    # xk2
                nc.vector.scalar_tensor_tensor(
                    gsb[:, k, :], xx2[:], c_cm[:, k, 1:2], ht[:, k, 1:TB + 1],
                    ALU.mult, ALU.add)        # xr2

            # ============ FFN ============
            kfA = big.tile([128, KC, TB], BF16, tag="mixt")  # alias mixt slot
            kfB = big.tile([128, 32, TB], BF16, tag="kfB")
            kfC = big.tile([128, 8, TB], BF16, tag="ht")     # alias ht slot

            def kf_view(i):
                if i < KC:
                    return kfA[:, i, :]
                return kfB[:, i - KC, :] if i < 48 else kfC[:, i - 48, :]

            for mf in range(KF):
                wt = wstr.tile([128, KC, 128], BF16, tag="wstream", bufs=3)
                nc.sync.dma_start(
                    wt[:], Wck_p[mf * 128:(mf + 1) * 128, :]
                    .rearrange("p (k f) -> p k f", k=KC))
                pt = pp(128, TB)
                for k in range(KC):
                    nc.tensor.matmul(pt[:], wt[:, k, :], xk2b[:, k, :],
                                     start=(k == 0), stop=(k == KC - 1))
                rl = sc.tile([128, TB], F32, tag="g1")
                nc.vector.tensor_scalar(rl[:], pt[:], 0.0, None, ALU.max)
                nc.scalar.activation(kf_view(mf), rl[:], AF.Square)

            for m in range(KC):
                ptu = pp(128, TB)
                for q in range(4):
                    wcv = wstr.tile([128, 14, 128], BF16, tag="wcv_s", bufs=2)
                    nc.sync.dma_start(
                        wcv[:], Wcv_p[m * 128:(m + 1) * 128,
                                      q * 14 * 128:(q + 1) * 14 * 128]
                        .rearrange("p (k f) -> p k f", k=14))
                    for kk in range(14):
                        ki = q * 14 + kk
                        nc.tensor.matmul(ptu[:], wcv[:, kk, :], kf_view(ki),
                                         start=(ki == 0), stop=(ki == KF - 1))
                wt = wstr.tile([128, KC, 128], BF16, tag="wstream", bufs=3)
                nc.sync.dma_start(
                    wt[:], Wp["cr"][m * 128:(m + 1) * 128, :]
                    .rearrange("p (k f) -> p k f", k=KC))
                pts = pp(128, TB)
                for k in range(KC):
                    nc.tensor.matmul(pts[:], wt[:, k, :], gsb[:, k, :],
                                     start=(k == 0), stop=(k == KC - 1))
                # sigmoid(x) = 1/(1+exp(-x)) via exp-table
                ssb = sc.tile([128, TB], F32, tag="g2")
                nc.scalar.activation(ssb[:], pts[:], AF.Exp, scale=-1.0)
                nc.vector.tensor_scalar(ssb[:], ssb[:], 1.0, None, ALU.add)
                nc.vector.reciprocal(ssb[:], ssb[:])
                ot = sc.tile([128, TB], F32, tag="g3", bufs=1)
                nc.vector.tensor_mul(ot[:], ptu[:], ssb[:])
                x2in = sc.tile([128, TB], F32, tag="g4", bufs=1)
                nc.sync.dma_start(x2in[:],
                                  x2d[128 * m:128 * (m + 1), 1:TB + 1])
                nc.vector.tensor_add(ot[:], ot[:], x2in[:])
                nc.sync.dma_start(outT[128 * m:128 * (m + 1), :], ot[:])

    nc.compile()
    return nc


_CACHE = {}


def _get_program():
    if "nc" not in _CACHE:
        _CACHE["nc"] = build_program()
    return _CACHE["nc"]


def _pret2(w):
    """Host pre-tile: out[m*128+p, k*128+f] = w[k*128+p, m*128+f]."""
    ci, co = w.shape
    KI, KO = ci // 128, co // 128
    return np.ascontiguousarray(
        w.reshape(KI, 128, KO, 128).transpose(2, 1, 0, 3)
    ).reshape(KO * 128, KI * 128)


def _shard_inputs(inp):
    f32 = np.float32
    x = np.asarray(inp["x"], f32)
    bf = lambda a: np.asarray(a, f32).astype(NP_BF16)

    maa_w2 = np.asarray(inp["maa_w2"], f32)         # (5, 32, C)
    maa_w2s = np.ascontiguousarray(
        maa_w2.reshape(5, 32, 4, 4, 128).transpose(0, 2, 1, 3, 4)
    ).reshape(5 * 4 * 32, 4 * 128)
    td_w2 = np.asarray(inp["td_w2"], f32)           # (64, C)
    Wv = np.asarray(inp["Wv"], f32)
    Wv_p = np.ascontiguousarray(
        Wv.reshape(4, 4, 128, 4, 512).transpose(3, 0, 2, 1, 4)
    ).reshape(4 * 4 * 128, 4 * 512)
    blk2 = np.zeros((128, 2), f32)
    blk2[0:64, 0] = 1.0
    blk2[64:128, 1] = 1.0

    shared = {
        "ln1_wb": np.stack([inp["ln1_w"], inp["ln1_b"]], 1).astype(f32),
        "ln2_wb": np.stack([inp["ln2_w"], inp["ln2_b"]], 1).astype(f32),
        "tm_maaT": np.asarray(inp["tm_maa"], f32).T.copy(),
        "cm_maaT": np.asarray(inp["cm_maa"], f32).T.copy(),
        "td_col": np.asarray(inp["time_decay"], f32).reshape(C, 1),
        "ident": np.eye(128, dtype=f32),
        "ident_bf": np.eye(128, dtype=f32).astype(NP_BF16),
        "mask_su": np.triu(np.ones((128, 128), f32), 1),
        "blk2h": blk2.astype(NP_BF16),
        "maa_w1": bf(inp["maa_w1"]),
        "maa_w2s": maa_w2s.astype(NP_BF16),
        "td_w1": bf(inp["td_w1"]),
        "td_w2f": td_w2.astype(NP_BF16),
        "Wr_p": bf(_pret2(np.asarray(inp["Wr"], f32))),
        "Wk_p": bf(_pret2(np.asarray(inp["Wk"], f32))),
        "Wg_p": bf(_pret2(np.asarray(inp["Wg"], f32))),
        "Wo_p": bf(_pret2(np.asarray(inp["Wo"], f32))),
        "Wcr_p": bf(_pret2(np.asarray(inp["Wcr"], f32))),
        "Wv_p": Wv_p.astype(NP_BF16),
        "Wck_p": bf(_pret2(np.asarray(inp["Wck"], f32))),
        "Wcv_p": bf(_pret2(np.asarray(inp["Wcv"], f32))),
    }
    u = np.asarray(inp["time_faaaa"], f32).reshape(C)
    lnx_w = np.asarray(inp["lnx_w"], f32)
    lnx_b = np.asarray(inp["lnx_b"], f32)

    in_maps = []
    for c in range(NCORE):
        b, blk = c // 4, c % 4
        ts = blk * TB
        xe = np.zeros((C, TB + 1), f32)
        xe[:, 1:] = x[b, ts:ts + TB].T
        if blk > 0:
            xe[:, 0] = x[b, ts - 1]
        ul = u[LCH * c:LCH * (c + 1)].reshape(2, 128).T.copy()
        base = LCH * c
        s2w = np.zeros((2, 2, 128), f32)
        for hp_ in range(2):
            wv_ = lnx_w[base + 128 * hp_:base + 128 * (hp_ + 1)]
            s2w[0, hp_, 0:64] = wv_[0:64]
            s2w[1, hp_, 64:128] = wv_[64:128]
        lnxc = np.stack([lnx_w[base:base + 128], lnx_w[base + 128:base + 256],
                         lnx_b[base:base + 128], lnx_b[base + 128:base + 256]],
                        1).astype(f32)
        sel = np.zeros((NCORE, 1), NP_BF16)
        if blk > 0:
            sel[c - 1, 0] = 1.0
        m = dict(shared)
        m.update({
            "xT": xe,
            "halo_mask": np.full((128, 1), 1.0 if blk > 0 else 0.0, f32),
            "sel_prev": sel,
            "u_loc": ul,
            "lnx_cols": lnxc,
            "sel2w": s2w.reshape(2, 256).astype(NP_BF16),
        })
        in_maps.append(m)
    return in_maps


def run(inputs, trace=False):
    nc = _get_program()
    in_maps = _shard_inputs(inputs)
    res = bass_utils.run_bass_kernel_spmd(
        nc, in_maps, core_ids=list(range(NCORE)), trace=trace)
    x = np.asarray(inputs["x"], np.float32)
    out = np.empty_like(x)
    for c in range(NCORE):
        b, blk = c // 4, c % 4
        out[b, blk * TB:(blk + 1) * TB, :] = np.asarray(
            res.results[c]["out"], np.float32).T
    return out, res.exec_time_ns


def kernel(**inputs):
    out, _ = run(inputs)
    return out


if __name__ == "__main__":
    build_program()
    print("build ok")
